# revision 1
# baseline (speedup 1.0000x reference)
"""Bass/Tile TRN2 kernel for nn_MultiHeadAttention_9277129359942.

B=2, T=S=2048, D=1024, H=16 heads, head_dim=64, fp32 I/O.

Sharding (8 cores): data-parallel over batch (2) x tensor-parallel over
head groups (4 heads / core, 256 out dims).  Each core computes the
attention for its 4 heads and a partial output projection; the host sums
the 4 bf16 partials per batch and adds the (linear) bo and bv terms
exactly: out = sum_g partial_g + bo + bv @ Wo.T.

v3 design notes:
  - Softmax exp is split across engines: head A of each pair uses the
    exact ACT exp (2 x N=512 chunks, started as soon as each score
    chunk lands), head B uses a one-instruction DVE fast-exp
    (Schraudolph: int16(x*EA+EC) bitcast as bf16, ~4% max rel err).
    End-to-end rel err 1.39e-2 (gate 2e-2), verified vs the reference.
  - Software pipelining in the attention loop: ctxA (ACT head) deferred
    one s-iteration; ctxB chunk0 same-iteration, chunk1 next iteration.
    Steady-state period ~1.5us/iter with PE/ACT/DVE all ~90% busy.
    PSUM: scA 2 + scB 2 + ctxA 2 + ctxB 2 = 8 banks.
  - Softmax denominators: the ones-column of v_aug makes row 64 of each
    ctx psum the denominator; 1/x runs on a [128,16] reshape (DVE
    reciprocal is ~6 cyc/elem per LANE, so a [1,1024] row costs 6.5us
    but [128,16] costs ~0.1us); the 64-partition broadcast is a log2
    SBUF DMA chain (last block: K=1 PE matmul so the tail never waits).
  - Normalize multiplies run on GpSimd (SBUF-only engine, otherwise
    idle); psum evictions and out-proj drains alternate DVE/ACT.
  - Inputs are DMA'd as full 128-partition tiles (engages all 16 SDMA
    engines) split across the sync and scalar HWDGE queues; q/k tiles
    first so the projections chase the loads.  Output is bf16 (halves
    the tail DMA); bo/bv are applied on the host (linear terms).
"""

import os
import sys

import numpy as np

for _p in ("/opt/trn_rl_repo",):
    if os.path.isdir(_p) and _p not in sys.path:
        sys.path.append(_p)

import ml_dtypes

import concourse.bass as bass
import concourse.mybir as mybir
import concourse.tile as tile
from concourse import bacc
from concourse.bass_utils import run_bass_kernel_spmd

F32 = mybir.dt.float32
BF16 = mybir.dt.bfloat16
I16 = mybir.dt.int16
AF = mybir.ActivationFunctionType
ALU = mybir.AluOpType
BF16_NP = ml_dtypes.bfloat16

D = 1024          # model dim
T = 2048          # query length
S = 2048          # key length
P = 128           # partitions
KT = D // P       # 8 contraction tiles
TT = T // P       # 16 row tiles
ST = S // P       # 16 key tiles
HL = 4            # local heads per core
HD = 64           # head dim
OUTL = HL * HD    # 256 local out dims
VW = HD + 1       # v_aug width per head (ones column appended)
N_CORES = 8

# fast-exp constants: exp(x*0.125) ~= bf16(bitcast(int16(x*EA + EC)))
EA = float(0.125 * 128.0 / np.log(2.0))
EC = float(127 * 128 - 7.5)
I32 = mybir.dt.int32
RMAGIC = 0x7EF311C3   # int-trick reciprocal seed constant


def build_program():
    """Build + compile the SPMD program (same on all 8 cores)."""
    nc = bacc.Bacc(
        "TRN2", target_bir_lowering=False, debug=False, enable_asserts=True,
        num_devices=N_CORES,
    )

    xq_d = nc.dram_tensor("xq", [D, T], BF16, kind="ExternalInput")
    xk_d = nc.dram_tensor("xk", [D, S], BF16, kind="ExternalInput")
    xv_d = nc.dram_tensor("xv", [D, S], BF16, kind="ExternalInput")
    wq_d = nc.dram_tensor("wq", [D, OUTL], BF16, kind="ExternalInput")
    wk_d = nc.dram_tensor("wk", [D, OUTL], BF16, kind="ExternalInput")
    wv_d = nc.dram_tensor("wv", [D, OUTL], BF16, kind="ExternalInput")
    wo_d = nc.dram_tensor("wo", [OUTL, D], BF16, kind="ExternalInput")
    bq_d = nc.dram_tensor("bq", [OUTL, 1], F32, kind="ExternalInput")
    bk_d = nc.dram_tensor("bk", [OUTL, 1], F32, kind="ExternalInput")
    out_d = nc.dram_tensor("out", [T, D], BF16, kind="ExternalOutput")
    wsink_d = nc.dram_tensor("warm_sink", [1, 8], F32, kind="ExternalOutput")

    with tile.TileContext(nc) as tc:
        _build(nc, tc, xq_d, xk_d, xv_d, wq_d, wk_d, wv_d, wo_d,
               bq_d, bk_d, out_d, wsink_d)
    nc.compile()
    return nc


def _build(nc, tc, xq_d, xk_d, xv_d, wq_d, wk_d, wv_d, wo_d,
           bq_d, bk_d, out_d, wsink_d):
    from contextlib import ExitStack

    stack = ExitStack()
    with stack:
        consts = stack.enter_context(tc.tile_pool(name="consts", bufs=1))
        wpool = stack.enter_context(tc.tile_pool(name="wpool", bufs=1))
        acts = stack.enter_context(tc.tile_pool(name="acts", bufs=1))


        bq_sb = consts.tile([P, 2], F32, name="bq", tag="bq")
        bk_sb = consts.tile([P, 2], F32, name="bk", tag="bk")
        wsnk = consts.tile([1, 8], F32, name="wsnk", tag="wsnk")

        wq_sb = [wpool.tile([P, OUTL], BF16, name=f"wq{k}", tag=f"wq{k}")
                 for k in range(KT)]
        wk_sb = [wpool.tile([P, OUTL], BF16, name=f"wk{k}", tag=f"wk{k}")
                 for k in range(KT)]
        wv_sb = [wpool.tile([P, OUTL], BF16, name=f"wv{k}", tag=f"wv{k}")
                 for k in range(KT)]
        wo_sb = [wpool.tile([P, D], BF16, name=f"wo{k}", tag=f"wo{k}")
                 for k in range(2)]

        qT = [acts.tile([P, T], BF16, name=f"qT{m}", tag=f"qT{m}")
              for m in range(2)]
        kT = [acts.tile([P, S], BF16, name=f"kT{m}", tag=f"kT{m}")
              for m in range(2)]
        v_aug = acts.tile([P, ST * HL * VW], BF16, name="vaug", tag="vaug")
        ctxT = [[acts.tile([P, 1024], BF16, name=f"ctxT{g}{th}",
                           tag=f"ctxT{g}{th}") for th in range(2)]
                for g in range(2)]

        xpool_cm = tc.tile_pool(name="xpool", bufs=1)
        xpool = xpool_cm.__enter__()
        xq_sb = [xpool.tile([P, T], BF16, name=f"xq{k}", tag=f"xq{k}")
                 for k in range(KT)]
        xk_sb = [xpool.tile([P, S], BF16, name=f"xk{k}", tag=f"xk{k}")
                 for k in range(KT)]
        xv_sb = [xpool.tile([P, S], BF16, name=f"xv{k}", tag=f"xv{k}")
                 for k in range(KT)]

        # Full 128-partition DMAs (all 16 SDMA engines per transfer),
        # alternating sync/scalar queues; q/k first, v later, wo last.
        nc.sync.dma_start(wq_sb[0][:], wq_d[0:P, :])
        nc.scalar.dma_start(wk_sb[0][:], wk_d[0:P, :])
        nc.sync.dma_start(bq_sb[:], bq_d.rearrange("(m p) o -> p (m o)", p=P))
        nc.scalar.dma_start(bk_sb[:], bk_d.rearrange("(m p) o -> p (m o)",
                                                     p=P))
        nc.sync.dma_start(xq_sb[0][:], xq_d[0:P, :])
        nc.scalar.dma_start(xk_sb[0][:], xk_d[0:P, :])
        for k in range(1, KT):
            nc.sync.dma_start(wq_sb[k][:], wq_d[k * P:(k + 1) * P, :])
            nc.scalar.dma_start(wk_sb[k][:], wk_d[k * P:(k + 1) * P, :])
            nc.sync.dma_start(xq_sb[k][:], xq_d[k * P:(k + 1) * P, :])
            nc.scalar.dma_start(xk_sb[k][:], xk_d[k * P:(k + 1) * P, :])
        for k in range(KT):
            eng = nc.sync if k % 2 == 0 else nc.scalar
            eng.dma_start(wv_sb[k][:], wv_d[k * P:(k + 1) * P, :])
        for k in range(KT):
            eng = nc.sync if k % 2 == 0 else nc.scalar
            eng.dma_start(xv_sb[k][:], xv_d[k * P:(k + 1) * P, :])
        for k in range(2):
            eng = nc.sync if k % 2 == 0 else nc.scalar
            eng.dma_start(wo_sb[k][:], wo_d[k * P:(k + 1) * P, :])

        nc.vector.memset(v_aug[:], 1.0)  # ones columns survive the v writes
        twos = consts.tile([1, 1024], F32, name="twos", tag="twos")
        nc.vector.memset(twos[:], 2.0)

        # ACT exp-table preload during the DMA head
        nc.scalar.activation(wsnk[0:1, 0:2], bq_sb[0:1, 0:2], AF.Exp)

        # ---- q/k projections: k-outer (chases the x DMAs) over two
        # t-half passes; all four (m, q/k) psum groups live at once -----
        with tc.tile_pool(name="qkpsum", bufs=1, space="PSUM") as qkpsum:
            # PE warmup burst (HAM un-throttle) during the DMA head
            warm = qkpsum.tile([P, 1024], F32, name="pq00", tag="pq0")
            for w in range(16):
                nc.tensor.matmul(warm[:, 0:OUTL], wq_sb[0][:, 0:P],
                                 wq_sb[0][:], start=(w == 0), stop=(w == 15))
            nc.vector.tensor_copy(wsnk[0:1, 2:4], warm[0:1, 0:2])
            nc.sync.dma_start(wsink_d[:, :], wsnk[:])

            groups = [(m, w_sb, x_sb, b_sb, o_sb)
                      for m in range(2)
                      for w_sb, x_sb, b_sb, o_sb in
                      ((wq_sb, xq_sb, bq_sb, qT),
                       (wk_sb, xk_sb, bk_sb, kT))]
            for tg in range(2):
                t_lo = tg * 1024
                ps_g = [qkpsum.tile([P, 1024], F32, name=f"pq{gi}{tg}",
                                    tag=f"pq{gi}")
                        for gi in range(4)]
                for k in range(KT):
                    for gi, (m, w_sb, x_sb, b_sb, o_sb) in enumerate(groups):
                        for c in range(2):
                            cs = slice(c * 512, (c + 1) * 512)
                            xs = slice(t_lo + c * 512, t_lo + (c + 1) * 512)
                            nc.tensor.matmul(
                                ps_g[gi][:, cs],
                                w_sb[k][:, m * P:(m + 1) * P],
                                x_sb[k][:, xs],
                                start=(k == 0), stop=(k == KT - 1))
                for gi, (m, w_sb, x_sb, b_sb, o_sb) in enumerate(groups):
                    nc.scalar.activation(
                        o_sb[m][:, t_lo:t_lo + 1024], ps_g[gi][:],
                        AF.Identity, bias=b_sb[:, m:m + 1])

        # ---- v projection (no bias: bv is applied on the host) ---------
        with tc.tile_pool(name="vpsum", bufs=2, space="PSUM") as vpsum:
            for s in range(ST):
                ps = vpsum.tile([P, OUTL], F32, name="pv", tag="pv")
                for k in range(KT):
                    nc.tensor.matmul(
                        ps[:], xv_sb[k][:, s * P:(s + 1) * P], wv_sb[k][:],
                        start=(k == 0), stop=(k == KT - 1))
                dst = v_aug[:, s * HL * VW:(s + 1) * HL * VW]
                dst = dst.rearrange("p (h x) -> p h x", x=VW)[:, :, 0:HD]
                nc.scalar.copy(dst, ps[:].rearrange("p (h x) -> p h x", x=HD))
        xpool_cm.__exit__(None, None, None)

        # ---- attention: 4 blocks of (head pair p, t-half th) -----------
        with tc.tile_pool(name="scpsum", bufs=1, space="PSUM") as scpsum, \
             tc.tile_pool(name="ctxpsum", bufs=1, space="PSUM") as ctxpsum, \
             tc.tile_pool(name="stgpool", bufs=3) as stgpool, \
             tc.tile_pool(name="nrmpool", bufs=3) as nrmpool, \
             tc.tile_pool(name="rbpool", bufs=2) as rbpool, \
             tc.tile_pool(name="epool", bufs=2) as epool:

            for p in range(2):          # head pair (local heads 2p, 2p+1)
                for th in range(2):     # t halves of 1024
                    t0 = th * 1024
                    ctxA = ctxpsum.tile([VW, 1024], F32, name="ctxA",
                                        tag="ctxA")
                    ctxB = ctxpsum.tile([VW, 1024], F32, name="ctxB",
                                        tag="ctxB")
                    hA = 2 * p
                    hB = 2 * p + 1

                    def vslice(h, s):
                        return slice(s * HL * VW + h * VW,
                                     s * HL * VW + (h + 1) * VW)

                    prevA = None   # (s, exA)
                    prevB = None   # (s, eiB)
                    for s in range(ST):
                        ss = slice(s * P, (s + 1) * P)
                        scA = scpsum.tile([P, 1024], F32, name="scA",
                                          tag="scA")
                        scB = scpsum.tile([P, 1024], F32, name="scB",
                                          tag="scB")
                        for c in range(2):
                            cs = slice(c * 512, (c + 1) * 512)
                            ts_ = slice(t0 + c * 512, t0 + (c + 1) * 512)
                            nc.tensor.matmul(scA[:, cs], kT[p][0:HD, ss],
                                             qT[p][0:HD, ts_],
                                             start=True, stop=True)
                            nc.tensor.matmul(scB[:, cs], kT[p][HD:P, ss],
                                             qT[p][HD:P, ts_],
                                             start=True, stop=True)
                        exA = epool.tile([P, 1024], BF16, name="exA",
                                         tag="exA")
                        eiB = epool.tile([P, 1024], I16, name="eiB",
                                         tag="eiB")
                        # DVE fast-exp, one instruction (per-instr overhead
                        # on the DVE is ~0.4us, so merged beats chunked)
                        nc.vector.tensor_scalar(eiB[:], scB[:],
                                                EA, EC, op0=ALU.mult,
                                                op1=ALU.add)
                        # ACT exact exp, chunked (starts after score chunk0)
                        nc.scalar.activation(exA[:, 0:512], scA[:, 0:512],
                                             AF.Exp, scale=0.125)
                        nc.scalar.activation(exA[:, 512:1024],
                                             scA[:, 512:1024],
                                             AF.Exp, scale=0.125)
                        # deferred ctxA/ctxB for iteration s-1
                        if prevA is not None:
                            sp, pexA = prevA
                            for c in range(2):
                                cs = slice(c * 512, (c + 1) * 512)
                                nc.tensor.matmul(
                                    ctxA[:, cs], v_aug[:, vslice(hA, sp)],
                                    pexA[:, cs],
                                    start=(sp == 0), stop=(sp == ST - 1))
                            sp, peiB = prevB
                            ebB = peiB[:].bitcast(BF16)
                            for c in range(2):
                                cs = slice(c * 512, (c + 1) * 512)
                                nc.tensor.matmul(
                                    ctxB[:, cs], v_aug[:, vslice(hB, sp)],
                                    ebB[:, cs],
                                    start=(sp == 0), stop=(sp == ST - 1))
                        prevA = (s, exA)
                        prevB = (s, eiB)
                    # drain the deferred tails
                    sp, pexA = prevA
                    sp2, peiB = prevB
                    for c in range(2):
                        cs = slice(c * 512, (c + 1) * 512)
                        nc.tensor.matmul(ctxA[:, cs],
                                         v_aug[:, vslice(hA, sp)],
                                         pexA[:, cs],
                                         start=(sp == 0), stop=True)
                    ebB = peiB[:].bitcast(BF16)
                    for c in range(2):
                        cs = slice(c * 512, (c + 1) * 512)
                        nc.tensor.matmul(ctxB[:, cs],
                                         v_aug[:, vslice(hB, sp2)],
                                         ebB[:, cs],
                                         start=(sp2 == 0), stop=True)

                    # evict ctx (DVE + ACT) and denom rows (DVE, to base
                    # partition 0); 1/denom = int-trick seed (DVE) + one
                    # Newton step; broadcast + normalize on GpSimd for
                    # blocks 0-2 (off the critical path), on the
                    # then-idle DVE for the last block.
                    last = (p, th) == (1, 1)
                    stgA = stgpool.tile([HD, 1024], F32, name="stgA",
                                        tag="stgA")
                    stgB = stgpool.tile([HD, 1024], F32, name="stgB",
                                        tag="stgB")
                    nc.vector.tensor_copy(stgA[:], ctxA[0:HD, :])
                    nc.vector.tensor_copy(stgB[:], ctxB[0:HD, :])
                    for i, (ctx, stg) in ((0, (ctxA, stgA)),
                                          (1, (ctxB, stgB))):
                        eng = nc.vector if last else nc.gpsimd
                        seed = nrmpool.tile([1, 1024], I32, name=f"sd{i}",
                                            tag=f"sd{i}")
                        nc.vector.tensor_scalar(
                            seed[:], ctx[HD:HD + 1, :].bitcast(I32),
                            -1, RMAGIC, op0=ALU.mult, op1=ALU.add)
                        r0f = seed[:].bitcast(F32)
                        e = nrmpool.tile([1, 1024], F32, name=f"e{i}",
                                         tag=f"e{i}")
                        nc.vector.tensor_tensor(out=e[:],
                                                in0=ctx[HD:HD + 1, :],
                                                in1=r0f, op=ALU.mult)
                        t_ = nrmpool.tile([1, 1024], F32, name=f"t{i}",
                                          tag=f"t{i}")
                        eng.tensor_tensor(out=t_[:], in0=twos[:],
                                          in1=e[:], op=ALU.subtract)
                        rn = nrmpool.tile([1, 1024], F32, name=f"rn{i}",
                                          tag=f"rn{i}")
                        eng.tensor_tensor(out=rn[:], in0=r0f,
                                          in1=t_[:], op=ALU.mult)
                        rb = rbpool.tile([HD, 1024], F32,
                                         name=f"rb{i}", tag=f"rb{i}")
                        # 64-partition broadcast: log2 DMA chain on the
                        # (idle) sync queue -- no engine involvement
                        nc.sync.dma_start(rb[0:1, :], rn[:])
                        w = 1
                        while w < HD:
                            nc.sync.dma_start(rb[w:2 * w, :], rb[0:w, :])
                            w *= 2
                        if i == 0:
                            eng.tensor_tensor(
                                out=ctxT[p][th][0:HD, :],
                                in0=stg[:], in1=rb[:],
                                op=ALU.mult)
                        else:
                            ostg = rbpool.tile([HD, 1024], BF16,
                                               name="ostg", tag="ostg")
                            eng.tensor_tensor(
                                out=ostg[:], in0=stg[:],
                                in1=rb[:], op=ALU.mult)
                            nc.sync.dma_start(
                                ctxT[p][th][HD:P, :], ostg[:])

        # ---- output projection -----------------------------------------
        with tc.tile_pool(name="popsum", bufs=2, space="PSUM") as popsum, \
             tc.tile_pool(name="opool", bufs=3) as opool:

            def emit_outproj(trange):
                for t in trange:
                    th_, tt_ = divmod(t, TT // 2)
                    ts_ = slice(tt_ * P, (tt_ + 1) * P)
                    po = popsum.tile([P, D], F32, name="po", tag="po")
                    for g in range(2):
                        for n in range(2):
                            ns = slice(n * 512, (n + 1) * 512)
                            nc.tensor.matmul(po[:, ns],
                                             ctxT[g][th_][:, ts_],
                                             wo_sb[g][:, ns],
                                             start=(g == 0), stop=(g == 1))
                    ost = opool.tile([P, D], BF16, name="ost", tag="ost")
                    if t % 2 == 0:
                        nc.vector.tensor_copy(ost[:], po[:])
                    else:
                        nc.scalar.copy(ost[:], po[:])
                    nc.sync.dma_start(out_d[t * P:(t + 1) * P, :], ost[:])

            emit_outproj(range(0, TT))


def make_in_maps(query, key, value, Wq, bq, Wk, bk, Wv, bv, Wo, bo):
    """Shard the full inputs into the 8 per-core input dicts."""
    query, key, value, Wq, bq, Wk, bk, Wv, bv, Wo, bo = [
        np.asarray(a, dtype=np.float32)
        for a in (query, key, value, Wq, bq, Wk, bk, Wv, bv, Wo, bo)]

    def bf(a):
        return np.ascontiguousarray(a).astype(BF16_NP)

    in_maps = []
    for c in range(N_CORES):
        b, g = divmod(c, 4)
        sl = slice(g * OUTL, (g + 1) * OUTL)
        in_maps.append({
            "xq": bf(query[b].T),
            "xk": bf(key[b].T),
            "xv": bf(value[b].T),
            "wq": bf(Wq[sl, :].T),
            "wk": bf(Wk[sl, :].T),
            "wv": bf(Wv[sl, :].T),
            "wo": bf(Wo[:, sl].T),
            "bq": np.ascontiguousarray(bq[sl].reshape(OUTL, 1)),
            "bk": np.ascontiguousarray(bk[sl].reshape(OUTL, 1)),
        })
    return in_maps


def gather_out(results, Wo, bo, bv):
    """Sum the per-core bf16 partials and add the host-side bias terms."""
    Wo = np.asarray(Wo, np.float32)
    bo = np.asarray(bo, np.float32)
    bv = np.asarray(bv, np.float32)
    host_bias = bo + bv @ Wo.T
    out = np.empty((2, T, D), dtype=np.float32)
    for b in range(2):
        acc = results[4 * b]["out"].astype(np.float32)
        for g in range(1, 4):
            acc = acc + results[4 * b + g]["out"].astype(np.float32)
        out[b] = acc + host_bias
    return out


_NC_CACHE = None


def _get_nc():
    global _NC_CACHE
    if _NC_CACHE is None:
        _NC_CACHE = build_program()
    return _NC_CACHE


def kernel(query, key, value, Wq, bq, Wk, bk, Wv, bv, Wo, bo):
    nc = _get_nc()
    in_maps = make_in_maps(query, key, value, Wq, bq, Wk, bk, Wv, bv, Wo, bo)
    res = run_bass_kernel_spmd(nc, in_maps, list(range(N_CORES))).results
    return gather_out(res, Wo, bo, bv)



# revision 4
# speedup vs baseline: 1.3331x; 1.3331x over previous
"""Bass/Tile TRN2 kernel for nn_MultiHeadAttention_9277129359942.

B=2, T=S=2048, D=1024, H=16 heads, head_dim=64, fp32 I/O.

Sharding (8 cores): data-parallel over batch (2) x tensor-parallel over
head groups (4 heads / core, 256 out dims).  Each core computes the
attention for its 4 heads and a partial output projection; the host sums
the 4 bf16 partials per batch and adds the (linear) bo and bv terms
exactly: out = sum_g partial_g + bo + bv @ Wo.T.

v4 design notes (vs v3 at ~315us):
  - ctx matmuls col-tile-packed: head A -> psum partitions 0-63
    (col groups 0,1), head B -> 64-127 (groups 2,3), concurrent on the
    PE array.  The ones-column denominator trick is replaced by four
    M=1 denominator matmuls per iteration, col-tiled to psum partitions
    0/32/64/96 of one shared bank - all four run concurrently, so the
    denominators cost ~1 matmul-stream instead of widening ctx to M=65.
  - exp split by t-chunk instead of by head: ACT exact exp on
    [scoresA-c0 | scoresB-c0] (one [128,1024] instr), DVE fast-exp
    (Schraudolph int16) on [A-c1 | B-c1].  Same fast fraction (1/2),
    same rel err (1.41e-2 predicted), but one instr per engine per
    iteration and psum banks free earlier.
  - blocks stream back-to-back on the PE (no per-block normalize stall):
    normalize runs on DVE/ACT/GpSimd + sync-queue DMA broadcast chains
    while the next block's matmuls proceed; HAM (PE clock throttle)
    stays warm instead of losing ~55us to half-clock windows.
  - last block's reciprocal broadcast via two concurrent K=1 PE matmuls
    into the freed score psum (no DMA-chain latency on the tail), then
    out-proj runs immediately; th=0 tiles are emitted first since they
    only depend on the first two blocks.
"""

import os
import sys

import numpy as np

for _p in ("/opt/trn_rl_repo",):
    if os.path.isdir(_p) and _p not in sys.path:
        sys.path.append(_p)

import ml_dtypes

import concourse.bass as bass
import concourse.mybir as mybir
import concourse.tile as tile
from concourse import bacc
from concourse.bass_utils import run_bass_kernel_spmd

F32 = mybir.dt.float32
BF16 = mybir.dt.bfloat16
I16 = mybir.dt.int16
AF = mybir.ActivationFunctionType
ALU = mybir.AluOpType
BF16_NP = ml_dtypes.bfloat16

D = 1024          # model dim
T = 2048          # query length
S = 2048          # key length
P = 128           # partitions
KT = D // P       # 8 contraction tiles
TT = T // P       # 16 row tiles
ST = S // P       # 16 key tiles
HL = 4            # local heads per core
HD = 64           # head dim
OUTL = HL * HD    # 256 local out dims
N_CORES = 8

# fast-exp constants: exp(x*0.125) ~= bf16(bitcast(int16(x*EA + EC)))
EA = float(0.125 * 128.0 / np.log(2.0))
EC = float(127 * 128 - 7.5)
I32 = mybir.dt.int32
RMAGIC = 0x7EF311C3   # int-trick reciprocal seed constant


def build_program():
    """Build + compile the SPMD program (same on all 8 cores)."""
    nc = bacc.Bacc(
        "TRN2", target_bir_lowering=False, debug=False, enable_asserts=True,
        num_devices=N_CORES,
    )

    xq_d = nc.dram_tensor("xq", [D, T], BF16, kind="ExternalInput")
    xk_d = nc.dram_tensor("xk", [D, S], BF16, kind="ExternalInput")
    xv_d = nc.dram_tensor("xv", [D, S], BF16, kind="ExternalInput")
    wq_d = nc.dram_tensor("wq", [D, OUTL], BF16, kind="ExternalInput")
    wk_d = nc.dram_tensor("wk", [D, OUTL], BF16, kind="ExternalInput")
    wv_d = nc.dram_tensor("wv", [D, OUTL], BF16, kind="ExternalInput")
    wo_d = nc.dram_tensor("wo", [OUTL, D], BF16, kind="ExternalInput")
    bq_d = nc.dram_tensor("bq", [OUTL, 1], F32, kind="ExternalInput")
    bk_d = nc.dram_tensor("bk", [OUTL, 1], F32, kind="ExternalInput")
    out_d = nc.dram_tensor("out", [T, D], BF16, kind="ExternalOutput")
    wsink_d = nc.dram_tensor("warm_sink", [1, 8], F32, kind="ExternalOutput")

    with tile.TileContext(nc) as tc:
        _build(nc, tc, xq_d, xk_d, xv_d, wq_d, wk_d, wv_d, wo_d,
               bq_d, bk_d, out_d, wsink_d)
    nc.compile()
    return nc


def _build(nc, tc, xq_d, xk_d, xv_d, wq_d, wk_d, wv_d, wo_d,
           bq_d, bk_d, out_d, wsink_d):
    from contextlib import ExitStack

    stack = ExitStack()
    with stack:
        consts = stack.enter_context(tc.tile_pool(name="consts", bufs=1))
        wpool = stack.enter_context(tc.tile_pool(name="wpool", bufs=1))
        acts = stack.enter_context(tc.tile_pool(name="acts", bufs=1))

        bq_sb = consts.tile([P, 2], F32, name="bq", tag="bq")
        bk_sb = consts.tile([P, 2], F32, name="bk", tag="bk")
        wsnk = consts.tile([1, 8], F32, name="wsnk", tag="wsnk")
        ones_sb = consts.tile([P, 1], BF16, name="ones", tag="ones")
        onesrow = consts.tile([P, HD], BF16, name="onesrow", tag="onesrow")

        wq_sb = [wpool.tile([P, OUTL], BF16, name=f"wq{k}", tag=f"wq{k}")
                 for k in range(KT)]
        wk_sb = [wpool.tile([P, OUTL], BF16, name=f"wk{k}", tag=f"wk{k}")
                 for k in range(KT)]
        wv_sb = [wpool.tile([P, OUTL], BF16, name=f"wv{k}", tag=f"wv{k}")
                 for k in range(KT)]
        wo_sb = [wpool.tile([P, D], BF16, name=f"wo{k}", tag=f"wo{k}")
                 for k in range(2)]

        qT = [acts.tile([P, T], BF16, name=f"qT{m}", tag=f"qT{m}")
              for m in range(2)]
        kT = [acts.tile([P, S], BF16, name=f"kT{m}", tag=f"kT{m}")
              for m in range(2)]
        v_sb = acts.tile([P, ST * HL * HD], BF16, name="vsb", tag="vsb")
        ctxT = [[acts.tile([P, 1024], BF16, name=f"ctxT{g}{th}",
                           tag=f"ctxT{g}{th}") for th in range(2)]
                for g in range(2)]

        xpool_cm = tc.tile_pool(name="xpool", bufs=1)
        xpool = xpool_cm.__enter__()
        xq_sb = [xpool.tile([P, T], BF16, name=f"xq{k}", tag=f"xq{k}")
                 for k in range(KT)]
        xk_sb = [xpool.tile([P, S], BF16, name=f"xk{k}", tag=f"xk{k}")
                 for k in range(KT)]
        xv_sb = [xpool.tile([P, S], BF16, name=f"xv{k}", tag=f"xv{k}")
                 for k in range(KT)]

        # Full 128-partition DMAs (all 16 SDMA engines per transfer),
        # alternating sync/scalar queues; q/k first, v later, wo last.
        nc.sync.dma_start(wq_sb[0][:], wq_d[0:P, :])
        nc.scalar.dma_start(wk_sb[0][:], wk_d[0:P, :])
        nc.sync.dma_start(bq_sb[:], bq_d.rearrange("(m p) o -> p (m o)", p=P))
        nc.scalar.dma_start(bk_sb[:], bk_d.rearrange("(m p) o -> p (m o)",
                                                     p=P))
        nc.sync.dma_start(xq_sb[0][:], xq_d[0:P, :])
        nc.scalar.dma_start(xk_sb[0][:], xk_d[0:P, :])
        for k in range(1, KT):
            nc.sync.dma_start(wq_sb[k][:], wq_d[k * P:(k + 1) * P, :])
            nc.scalar.dma_start(wk_sb[k][:], wk_d[k * P:(k + 1) * P, :])
            nc.sync.dma_start(xq_sb[k][:], xq_d[k * P:(k + 1) * P, :])
            nc.scalar.dma_start(xk_sb[k][:], xk_d[k * P:(k + 1) * P, :])
        for k in range(KT):
            eng = nc.sync if k % 2 == 0 else nc.scalar
            eng.dma_start(wv_sb[k][:], wv_d[k * P:(k + 1) * P, :])
        for k in range(KT):
            eng = nc.sync if k % 2 == 0 else nc.scalar
            eng.dma_start(xv_sb[k][:], xv_d[k * P:(k + 1) * P, :])
        for k in range(2):
            eng = nc.sync if k % 2 == 0 else nc.scalar
            eng.dma_start(wo_sb[k][:], wo_d[k * P:(k + 1) * P, :])

        nc.vector.memset(ones_sb[:], 1.0)
        nc.vector.memset(onesrow[:], 1.0)

        # ACT exp-table preload during the DMA head
        nc.scalar.activation(wsnk[0:1, 0:2], bq_sb[0:1, 0:2], AF.Exp)

        # ---- q/k projections: k-outer (chases the x DMAs) over two
        # t-half passes; all four (m, q/k) psum groups live at once -----
        with tc.tile_pool(name="qkpsum", bufs=1, space="PSUM") as qkpsum:
            # PE warmup burst (HAM un-throttle) during the DMA head
            warm = qkpsum.tile([P, 1024], F32, name="pq00", tag="pq0")
            for w in range(16):
                nc.tensor.matmul(warm[:, 0:OUTL], wq_sb[0][:, 0:P],
                                 wq_sb[0][:], start=(w == 0), stop=(w == 15))
            nc.vector.tensor_copy(wsnk[0:1, 2:4], warm[0:1, 0:2])
            nc.sync.dma_start(wsink_d[:, :], wsnk[:])

            groups = [(m, w_sb, x_sb, b_sb, o_sb)
                      for m in range(2)
                      for w_sb, x_sb, b_sb, o_sb in
                      ((wq_sb, xq_sb, bq_sb, qT),
                       (wk_sb, xk_sb, bk_sb, kT))]
            for tg in range(2):
                t_lo = tg * 1024
                ps_g = [qkpsum.tile([P, 1024], F32, name=f"pq{gi}{tg}",
                                    tag=f"pq{gi}")
                        for gi in range(4)]
                for k in range(KT):
                    for gi, (m, w_sb, x_sb, b_sb, o_sb) in enumerate(groups):
                        for c in range(2):
                            cs = slice(c * 512, (c + 1) * 512)
                            xs = slice(t_lo + c * 512, t_lo + (c + 1) * 512)
                            nc.tensor.matmul(
                                ps_g[gi][:, cs],
                                w_sb[k][:, m * P:(m + 1) * P],
                                x_sb[k][:, xs],
                                start=(k == 0), stop=(k == KT - 1))
                for gi, (m, w_sb, x_sb, b_sb, o_sb) in enumerate(groups):
                    nc.scalar.activation(
                        o_sb[m][:, t_lo:t_lo + 1024], ps_g[gi][:],
                        AF.Identity, bias=b_sb[:, m:m + 1])

        # ---- v projection (no bias: bv is applied on the host) ---------
        with tc.tile_pool(name="vpsum", bufs=2, space="PSUM") as vpsum:
            for s in range(ST):
                ps = vpsum.tile([P, OUTL], F32, name="pv", tag="pv")
                for k in range(KT):
                    nc.tensor.matmul(
                        ps[:], xv_sb[k][:, s * P:(s + 1) * P], wv_sb[k][:],
                        start=(k == 0), stop=(k == KT - 1))
                nc.scalar.copy(v_sb[:, s * OUTL:(s + 1) * OUTL], ps[:])
        xpool_cm.__exit__(None, None, None)

        # ---- attention: 4 blocks of (head pair p, t-half th), streamed
        # back-to-back on the PE -----------------------------------------
        with tc.tile_pool(name="scpsum", bufs=1, space="PSUM") as scpsum, \
             tc.tile_pool(name="ctxpsum", bufs=1, space="PSUM") as ctxpsum, \
             tc.tile_pool(name="dnpsum", bufs=2, space="PSUM") as dnpsum, \
             tc.tile_pool(name="stgpool", bufs=2) as stgpool, \
             tc.tile_pool(name="nrmpool", bufs=2) as nrmpool, \
             tc.tile_pool(name="rbpool", bufs=2) as rbpool, \
             tc.tile_pool(name="epool", bufs=2) as epool:

            # block order: th=0 blocks first so the th=0 out-proj tiles
            # are unblocked long before the tail
            blocks = [(0, 0), (1, 0), (0, 1), (1, 1)]
            for bi, (p, th) in enumerate(blocks):
                t0 = th * 1024
                last = bi == len(blocks) - 1
                hA = 2 * p
                hB = 2 * p + 1
                ctxAB = ctxpsum.tile([P, 1024], F32, name="ctxAB",
                                     tag="ctxAB")
                dn = dnpsum.tile([P, 512], F32, name="dn", tag="dn")

                def vsl(h, s):
                    return slice(s * OUTL + h * HD, s * OUTL + (h + 1) * HD)

                prev = None
                sc23_t = None
                for s in range(ST):
                    ss = slice(s * P, (s + 1) * P)
                    sc01 = scpsum.tile([P, 1024], F32, name="sc01",
                                       tag="sc01")
                    sc23 = scpsum.tile([P, 1024], F32, name="sc23",
                                       tag="sc23")
                    sc23_t = sc23
                    # scores: c0 pair then c1 pair; A (rows 0-63) and B
                    # (rows 64-127) run concurrently via PE row tiling
                    nc.tensor.matmul(sc01[:, 0:512], kT[p][0:HD, ss],
                                     qT[p][0:HD, t0:t0 + 512],
                                     start=True, stop=True)
                    nc.tensor.matmul(sc01[:, 512:1024], kT[p][HD:P, ss],
                                     qT[p][HD:P, t0:t0 + 512],
                                     start=True, stop=True)
                    nc.tensor.matmul(sc23[:, 0:512], kT[p][0:HD, ss],
                                     qT[p][0:HD, t0 + 512:t0 + 1024],
                                     start=True, stop=True)
                    nc.tensor.matmul(sc23[:, 512:1024], kT[p][HD:P, ss],
                                     qT[p][HD:P, t0 + 512:t0 + 1024],
                                     start=True, stop=True)
                    # exp: ACT exact on [A-c0|B-c0], DVE fast on [A-c1|B-c1]
                    e0 = epool.tile([P, 1024], BF16, name="e0", tag="e0")
                    e1 = epool.tile([P, 1024], I16, name="e1", tag="e1")
                    nc.scalar.activation(e0[:], sc01[:], AF.Exp, scale=0.125)
                    nc.vector.tensor_scalar(e1[:], sc23[:], EA, EC,
                                            op0=ALU.mult, op1=ALU.add)
                    # deferred ctx + denominators for iteration s-1
                    if prev is not None:
                        sp, pe0, pe1 = prev
                        pe1b = pe1[:].bitcast(BF16)
                        st_, so_ = (sp == 0), (sp == ST - 1)
                        vA = v_sb[:, vsl(hA, sp)]
                        vB = v_sb[:, vsl(hB, sp)]
                        nc.tensor.matmul(ctxAB[0:HD, 0:512], vA,
                                         pe0[:, 0:512], start=st_, stop=so_)
                        nc.tensor.matmul(ctxAB[HD:P, 0:512], vB,
                                         pe0[:, 512:1024],
                                         start=st_, stop=so_)
                        nc.tensor.matmul(ctxAB[0:HD, 512:1024], vA,
                                         pe1b[:, 0:512], start=st_, stop=so_)
                        nc.tensor.matmul(ctxAB[HD:P, 512:1024], vB,
                                         pe1b[:, 512:1024],
                                         start=st_, stop=so_)
                        nc.tensor.matmul(dn[0:1, :], ones_sb[:],
                                         pe0[:, 0:512], start=st_, stop=so_,
                                         tile_position=(0, 0))
                        nc.tensor.matmul(dn[32:33, :], ones_sb[:],
                                         pe1b[:, 0:512], start=st_, stop=so_,
                                         tile_position=(0, 32))
                        nc.tensor.matmul(dn[64:65, :], ones_sb[:],
                                         pe0[:, 512:1024],
                                         start=st_, stop=so_,
                                         tile_position=(0, 64))
                        nc.tensor.matmul(dn[96:97, :], ones_sb[:],
                                         pe1b[:, 512:1024],
                                         start=st_, stop=so_,
                                         tile_position=(0, 96))
                    prev = (s, e0, e1)
                # drain the deferred tail (s = ST-1)
                sp, pe0, pe1 = prev
                pe1b = pe1[:].bitcast(BF16)
                vA = v_sb[:, vsl(hA, sp)]
                vB = v_sb[:, vsl(hB, sp)]
                nc.tensor.matmul(ctxAB[0:HD, 0:512], vA, pe0[:, 0:512],
                                 start=False, stop=True)
                nc.tensor.matmul(ctxAB[HD:P, 0:512], vB, pe0[:, 512:1024],
                                 start=False, stop=True)
                nc.tensor.matmul(ctxAB[0:HD, 512:1024], vA, pe1b[:, 0:512],
                                 start=False, stop=True)
                nc.tensor.matmul(ctxAB[HD:P, 512:1024], vB,
                                 pe1b[:, 512:1024], start=False, stop=True)
                nc.tensor.matmul(dn[0:1, :], ones_sb[:], pe0[:, 0:512],
                                 start=False, stop=True,
                                 tile_position=(0, 0))
                nc.tensor.matmul(dn[32:33, :], ones_sb[:], pe1b[:, 0:512],
                                 start=False, stop=True,
                                 tile_position=(0, 32))
                nc.tensor.matmul(dn[64:65, :], ones_sb[:], pe0[:, 512:1024],
                                 start=False, stop=True,
                                 tile_position=(0, 64))
                nc.tensor.matmul(dn[96:97, :], ones_sb[:],
                                 pe1b[:, 512:1024], start=False, stop=True,
                                 tile_position=(0, 96))

                # evict ctx psum (ACT) so the next block's ctx can start
                stg = stgpool.tile([P, 1024], F32, name="stg", tag="stg")
                nc.scalar.copy(stg[:], ctxAB[:])

                # 1/denom: int-trick seed + one Newton step.  dn rows
                # {0,32,64,96} hold [A-c0, A-c1, B-c0, B-c1] sums.
                seed = nrmpool.tile([P, 512], I32, name="seed", tag="seed")
                nc.vector.tensor_scalar(seed[:], dn[:].bitcast(I32),
                                        -1, RMAGIC, op0=ALU.mult,
                                        op1=ALU.add)
                r0f = seed[:].bitcast(F32)
                e_ = nrmpool.tile([P, 512], F32, name="e_", tag="e_")
                nc.vector.tensor_tensor(out=e_[:], in0=dn[:], in1=r0f,
                                        op=ALU.mult)
                t_ = nrmpool.tile([P, 512], F32, name="t_", tag="t_")
                nc.vector.tensor_scalar(t_[:], e_[:], -1.0, 2.0,
                                        op0=ALU.mult, op1=ALU.add)
                rn = nrmpool.tile([P, 512], F32, name="rn", tag="rn")
                nc.vector.tensor_tensor(out=rn[:], in0=r0f, in1=t_[:],
                                        op=ALU.mult)

                if not last:
                    # broadcast recip rows to a [128,1024] rb via log2
                    # DMA chains on the (idle) sync queue, then normalize
                    # on GpSimd - all off the PE critical path.
                    rb = rbpool.tile([P, 1024], F32, name="rb", tag="rb")
                    nc.sync.dma_start(rb[0:1, 0:512], rn[0:1, :])
                    nc.sync.dma_start(rb[0:1, 512:1024], rn[32:33, :])
                    nc.sync.dma_start(rb[HD:HD + 1, 0:512], rn[64:65, :])
                    nc.sync.dma_start(rb[HD:HD + 1, 512:1024], rn[96:97, :])
                    w = 1
                    while w < HD:
                        nc.sync.dma_start(rb[w:2 * w, :], rb[0:w, :])
                        nc.sync.dma_start(rb[HD + w:HD + 2 * w, :],
                                          rb[HD:HD + w, :])
                        w *= 2
                    nc.gpsimd.tensor_tensor(out=ctxT[p][th][:], in0=stg[:],
                                            in1=rb[:], op=ALU.mult)
                else:
                    # tail: bf16 recip rows -> 2 concurrent K=1 PE
                    # broadcast matmuls into the freed score psum, then a
                    # single DVE normalize.  No DMA-chain latency.
                    rnb = rbpool.tile([P, 512], BF16, name="rnb", tag="rnb")
                    nc.vector.tensor_copy(rnb[:], rn[:])
                    flt = rbpool.tile([P, 1024], BF16, name="flt", tag="flt")
                    nc.sync.dma_start(flt[0:1, 0:512], rnb[0:1, :])
                    nc.sync.dma_start(flt[0:1, 512:1024], rnb[32:33, :])
                    nc.sync.dma_start(flt[HD:HD + 1, 0:512], rnb[64:65, :])
                    nc.sync.dma_start(flt[HD:HD + 1, 512:1024],
                                      rnb[96:97, :])
                    rbp = scpsum.tile([P, 1024], F32, name="rbp",
                                      tag="sc23")
                    nc.tensor.matmul(rbp[0:HD, 0:512], onesrow[0:1, :],
                                     flt[0:1, 0:512], start=True, stop=True)
                    nc.tensor.matmul(rbp[0:HD, 512:1024], onesrow[0:1, :],
                                     flt[0:1, 512:1024],
                                     start=True, stop=True)
                    nc.tensor.matmul(rbp[HD:P, 0:512], onesrow[HD:HD + 1, :],
                                     flt[HD:HD + 1, 0:512],
                                     start=True, stop=True)
                    nc.tensor.matmul(rbp[HD:P, 512:1024],
                                     onesrow[HD:HD + 1, :],
                                     flt[HD:HD + 1, 512:1024],
                                     start=True, stop=True)
                    nc.vector.tensor_tensor(out=ctxT[p][th][:], in0=stg[:],
                                            in1=rbp[:], op=ALU.mult)

        # ---- output projection -----------------------------------------
        with tc.tile_pool(name="popsum", bufs=2, space="PSUM") as popsum, \
             tc.tile_pool(name="opool", bufs=3) as opool:

            def emit_outproj(trange):
                for t in trange:
                    th_, tt_ = divmod(t, TT // 2)
                    ts_ = slice(tt_ * P, (tt_ + 1) * P)
                    po = popsum.tile([P, D], F32, name="po", tag="po")
                    for g in range(2):
                        for n in range(2):
                            ns = slice(n * 512, (n + 1) * 512)
                            nc.tensor.matmul(po[:, ns],
                                             ctxT[g][th_][:, ts_],
                                             wo_sb[g][:, ns],
                                             start=(g == 0), stop=(g == 1))
                    ost = opool.tile([P, D], BF16, name="ost", tag="ost")
                    if t % 2 == 0:
                        nc.vector.tensor_copy(ost[:], po[:])
                    else:
                        nc.scalar.copy(ost[:], po[:])
                    eng = nc.sync if t % 2 == 0 else nc.scalar
                    eng.dma_start(out_d[t * P:(t + 1) * P, :], ost[:])

            emit_outproj(range(0, TT))


def make_in_maps(query, key, value, Wq, bq, Wk, bk, Wv, bv, Wo, bo):
    """Shard the full inputs into the 8 per-core input dicts."""
    query, key, value, Wq, bq, Wk, bk, Wv, bv, Wo, bo = [
        np.asarray(a, dtype=np.float32)
        for a in (query, key, value, Wq, bq, Wk, bk, Wv, bv, Wo, bo)]

    def bf(a):
        return np.ascontiguousarray(a).astype(BF16_NP)

    in_maps = []
    for c in range(N_CORES):
        b, g = divmod(c, 4)
        sl = slice(g * OUTL, (g + 1) * OUTL)
        in_maps.append({
            "xq": bf(query[b].T),
            "xk": bf(key[b].T),
            "xv": bf(value[b].T),
            "wq": bf(Wq[sl, :].T),
            "wk": bf(Wk[sl, :].T),
            "wv": bf(Wv[sl, :].T),
            "wo": bf(Wo[:, sl].T),
            "bq": np.ascontiguousarray(bq[sl].reshape(OUTL, 1)),
            "bk": np.ascontiguousarray(bk[sl].reshape(OUTL, 1)),
        })
    return in_maps


def gather_out(results, Wo, bo, bv):
    """Sum the per-core bf16 partials and add the host-side bias terms."""
    Wo = np.asarray(Wo, np.float32)
    bo = np.asarray(bo, np.float32)
    bv = np.asarray(bv, np.float32)
    host_bias = bo + bv @ Wo.T
    out = np.empty((2, T, D), dtype=np.float32)
    for b in range(2):
        acc = results[4 * b]["out"].astype(np.float32)
        for g in range(1, 4):
            acc = acc + results[4 * b + g]["out"].astype(np.float32)
        out[b] = acc + host_bias
    return out


_NC_CACHE = None


def _get_nc():
    global _NC_CACHE
    if _NC_CACHE is None:
        _NC_CACHE = build_program()
    return _NC_CACHE


def kernel(query, key, value, Wq, bq, Wk, bk, Wv, bv, Wo, bo):
    nc = _get_nc()
    in_maps = make_in_maps(query, key, value, Wq, bq, Wk, bk, Wv, bv, Wo, bo)
    res = run_bass_kernel_spmd(nc, in_maps, list(range(N_CORES))).results
    return gather_out(res, Wo, bo, bv)
